# revision 9
# baseline (speedup 1.0000x reference)
"""Trainium2 Bass kernel for nn_EnsembleNet (10-head MLP ensemble).

Math (per head h):
  h1 = relu(x @ W1[h] + b1[h])      x: [B, 129], W1: [129, 16]
  h2 = relu(h1 @ W2[h] + b2[h])     W2: [16, 16]
  out[h] = h2 @ W3[h] + b3[h]       W3: [16, 16] -> [10, B, 16]

Strategy (data parallel over 8 cores, B=500000 -> 62500/core, padded to
63488 = 124 tiles x 512 samples):
  - Host splits x into xm=[B,128] (features 0..127) and xl=[1,B] (feature
    128) so the 129-feature contraction becomes one K=128 matmul plus one
    K=1 accumulate whose moving operand loads directly as a [1, 512] row.
  - Heads 0-7 ("A", 8*16=128 outputs) computed in the transposed domain:
    PE-transpose x tiles to [128 feat, 512 batch], then chained block-
    diagonal matmuls (float32r, 1 cyc/row) with fused bias+relu on ACT,
    PE-transpose the result back to batch-major and DMA out.
  - Heads 8-9 ("B", 32 outputs) are packed 4 sub-tiles deep on the
    partition axis (4 x 32 = 128) per 2048-sample super-tile so vector
    ops and matmuls run at full width; layer 1 uses column-group packing
    (tile_position via psum base partition) to place each sub-tile's
    [K,32] result in its own 32-partition strip.
  - Device writes custom layouts outA [496,128,128] / outB [31,4,128,128];
    host permutes (64B-granular) into [10, B, 16].
"""

import os
from contextlib import ExitStack

import numpy as np

import concourse.bass as bass
import concourse.mybir as mybir
import concourse.tile as tile
from concourse import bacc
from concourse.bass_utils import run_bass_kernel_spmd
from concourse.masks import make_identity

F32 = mybir.dt.float32
F32R = mybir.dt.float32r

N_CORES = 8
BATCH = 500000
SHARD = BATCH // N_CORES  # 62500
TILE = 512  # samples per tile (psum bank = 512 f32)
SUPER = 4 * TILE  # 2048, B-group packing unit
N_TILES = 124  # ceil(62500/512) -> pad to 124
PAD = N_TILES * TILE  # 63488
N_SUPERS = N_TILES // 4  # 31

NHEADS = 10
HID = 16
SKIP = 16
IN_DIM = 129


def _block_diag(mats):
    n = len(mats)
    r, c = mats[0].shape
    out = np.zeros((n * r, n * c), dtype=mats[0].dtype)
    for i, m in enumerate(mats):
        out[i * r : (i + 1) * r, i * c : (i + 1) * c] = m
    return out


def _pack_weights(W1, b1, W2, b2, W3, b3):
    """Host-side packing into the SBUF layouts the kernel expects."""
    W1 = np.asarray(W1, np.float32)
    W2 = np.asarray(W2, np.float32)
    W3 = np.asarray(W3, np.float32)
    b1 = np.asarray(b1, np.float32)
    b2 = np.asarray(b2, np.float32)
    b3 = np.asarray(b3, np.float32)

    d = {}
    # L1 A: lhsT [K=128 feat, M=128 (h,o)]
    d["w1a"] = np.ascontiguousarray(W1[:8, :128, :].transpose(1, 0, 2).reshape(128, 128))
    d["wla"] = np.ascontiguousarray(W1[:8, 128, :].reshape(1, 128))
    # L1 B: zero-padded M=128 variants, chunk c owns columns c*32..(c+1)*32
    # (fp32r matmul requires dst psum base partition 0, so each chunk's
    # [K,32] result is placed via its weight columns instead of col-tiling)
    w1b32 = W1[8:, :128, :].transpose(1, 0, 2).reshape(128, 32)
    wlb32 = W1[8:, 128, :].reshape(32)
    w1bs = np.zeros((4, 128, 128), np.float32)
    wlbs = np.zeros((4, 128), np.float32)
    for c in range(4):
        w1bs[c, :, c * 32 : (c + 1) * 32] = w1b32
        wlbs[c, c * 32 : (c + 1) * 32] = wlb32
    d["w1bs"] = w1bs
    d["wlbs"] = wlbs
    # L2: block diag [in (h,i), out (h,o)]
    d["w2a"] = _block_diag([W2[h] for h in range(8)])
    w2b1 = _block_diag([W2[8], W2[9]])  # [32, 32]
    d["w2b"] = _block_diag([w2b1] * 4)  # [128, 128] over (c, g)
    d["w3a"] = _block_diag([W3[h] for h in range(8)])
    w3b1 = _block_diag([W3[8], W3[9]])
    d["w3b"] = _block_diag([w3b1] * 4)
    # biases, per-partition [128, 1]
    d["b1a"] = b1[:8].reshape(128, 1).copy()
    d["b1b"] = np.tile(b1[8:].reshape(-1), 4).reshape(128, 1)
    d["b2a"] = b2[:8].reshape(128, 1).copy()
    d["b2b"] = np.tile(b2[8:].reshape(-1), 4).reshape(128, 1)
    d["b3a"] = b3[:8].reshape(128, 1).copy()
    d["b3b"] = np.tile(b3[8:].reshape(-1), 4).reshape(128, 1)
    return {k: np.ascontiguousarray(v, dtype=np.float32) for k, v in d.items()}


def _kernel_body(tc, outs, ins):
    nc = tc.nc
    outA, outB = outs["outA"], outs["outB"]  # [496,128,128], [31,4,128,128]
    xm, xl = ins["xm"], ins["xl"]  # [PAD, 128], [1, PAD]
    relu = mybir.ActivationFunctionType.Relu
    copyf = mybir.ActivationFunctionType.Copy

    with ExitStack() as ctx:
        const = ctx.enter_context(tc.tile_pool(name="const", bufs=1))

        ident = const.tile([128, 128], F32)
        make_identity(nc, ident)

        def ld(name, shape, dt=F32):
            t = const.tile(shape, dt, name=name)
            nc.sync.dma_start(t, ins[name])
            return t

        w1a = ld("w1a", [128, 128], F32R)
        wla = ld("wla", [1, 128], F32R)
        w1bs = [None] * 4
        wlbs = [None] * 4
        for c in range(4):
            w1bs[c] = const.tile([128, 128], F32R, name=f"w1bs{c}")
            nc.sync.dma_start(w1bs[c], ins["w1bs"][c])
            wlbs[c] = const.tile([1, 128], F32R, name=f"wlbs{c}")
            nc.sync.dma_start(wlbs[c], ins["wlbs"][c : c + 1, :])
        w2a = ld("w2a", [128, 128], F32R)
        w2b = ld("w2b", [128, 128], F32R)
        w3a = ld("w3a", [128, 128], F32R)
        w3b = ld("w3b", [128, 128], F32R)
        b1a = ld("b1a", [128, 1])
        b1b = ld("b1b", [128, 1])
        b2a = ld("b2a", [128, 1])
        b2b = ld("b2b", [128, 1])
        b3a = ld("b3a", [128, 1])
        b3b = ld("b3b", [128, 1])

        io_pool = ctx.enter_context(tc.tile_pool(name="io", bufs=3))
        xt_pool = ctx.enter_context(tc.tile_pool(name="xt", bufs=6))
        h_pool = ctx.enter_context(tc.tile_pool(name="h", bufs=3))
        bsb_pool = ctx.enter_context(tc.tile_pool(name="bsb", bufs=2))
        xl_pool = ctx.enter_context(tc.tile_pool(name="xlp", bufs=2))
        ppool = ctx.enter_context(tc.tile_pool(name="ppool", space="PSUM", bufs=2))
        papool = ctx.enter_context(tc.tile_pool(name="papool", space="PSUM", bufs=2))
        pbpool = ctx.enter_context(tc.tile_pool(name="pbpool", space="PSUM", bufs=2))
        ptpool = ctx.enter_context(tc.tile_pool(name="ptpool", space="PSUM", bufs=2))

        for s in range(N_SUPERS):
            # xlast row for this super-tile: [1, 2048] straight from DRAM
            xlt = xl_pool.tile([1, SUPER], F32R, tag="xlt")
            nc.sync.dma_start(xlt, xl[:, s * SUPER : (s + 1) * SUPER])

            pb1 = pbpool.tile([128, TILE], F32, tag="pb")
            xts = []
            for c in range(4):
                t = s * 4 + c
                # natural load: [128 part, (c4, f128)]
                xm_t = io_pool.tile([128, 512], F32, tag="xm")
                src = xm[t * TILE : (t + 1) * TILE, :].rearrange(
                    "(c p) f -> p c f", p=128
                )
                nc.sync.dma_start(xm_t.rearrange("p (c f) -> p c f", c=4), src)

                # transpose 4 chunks of [128,128] -> [feat, batch]
                px = ppool.tile([128, TILE], F32, tag="px")
                for j in range(4):
                    nc.tensor.transpose(
                        px[:, j * 128 : (j + 1) * 128],
                        xm_t[:, j * 128 : (j + 1) * 128],
                        ident,
                    )
                xt = xt_pool.tile([128, TILE], F32R, tag="xt")
                nc.scalar.activation(xt, px, copyf)
                xts.append(xt)

                xls = xlt[:, c * TILE : (c + 1) * TILE]

                # ---- A group ----
                pa1 = papool.tile([128, TILE], F32, tag="pa")
                nc.tensor.matmul(
                    pa1, w1a, xt, start=True, stop=False
                )
                nc.tensor.matmul(
                    pa1, wla, xls, start=False, stop=True
                )
                h1a = h_pool.tile([128, TILE], F32R, tag="h1a")
                nc.scalar.activation(h1a, pa1, relu, bias=b1a)

                pa2 = papool.tile([128, TILE], F32, tag="pa")
                nc.tensor.matmul(
                    pa2, w2a, h1a, start=True, stop=True
                )
                h2a = h_pool.tile([128, TILE], F32R, tag="h2a")
                nc.scalar.activation(h2a, pa2, relu, bias=b2a)

                pa3 = papool.tile([128, TILE], F32, tag="pa")
                nc.tensor.matmul(
                    pa3, w3a, h2a, start=True, stop=True
                )
                outta = h_pool.tile([128, TILE], F32, tag="outta")
                nc.vector.tensor_scalar_add(outta, pa3, b3a)

                # transpose back to batch-major and store
                pt = ptpool.tile([128, TILE], F32, tag="pt")
                for j in range(4):
                    nc.tensor.transpose(
                        pt[:, j * 128 : (j + 1) * 128],
                        outta[:, j * 128 : (j + 1) * 128],
                        ident,
                    )
                oa = io_pool.tile([128, TILE], F32, tag="oa")
                nc.vector.tensor_copy(oa, pt)
                for j in range(4):
                    nc.sync.dma_start(
                        outA[t * 4 + j, :, :], oa[:, j * 128 : (j + 1) * 128]
                    )

                # ---- B group layer 1: full-M matmul with zero-padded
                # weights accumulating chunk c's strip into packed psum ----
                nc.tensor.matmul(pb1, w1bs[c], xt, start=(c == 0), stop=False)
                nc.tensor.matmul(pb1, wlbs[c], xls, start=False, stop=(c == 3))

            # ---- B group layers 2..3, packed [128=(c,g,i), 512] ----
            h1b = bsb_pool.tile([128, TILE], F32R, tag="h1b")
            nc.scalar.activation(h1b, pb1, relu, bias=b1b)

            pb2 = pbpool.tile([128, TILE], F32, tag="pb")
            nc.tensor.matmul(
                pb2, w2b, h1b, start=True, stop=True
            )
            h2b = bsb_pool.tile([128, TILE], F32R, tag="h2b")
            nc.scalar.activation(h2b, pb2, relu, bias=b2b)

            pb3 = pbpool.tile([128, TILE], F32, tag="pb")
            nc.tensor.matmul(
                pb3, w3b, h2b, start=True, stop=True
            )
            outtb = bsb_pool.tile([128, TILE], F32, tag="outtb")
            nc.vector.tensor_scalar_add(outtb, pb3, b3b)

            for j in range(4):
                ptb = ptpool.tile([128, 128], F32, tag="pt")
                nc.tensor.transpose(ptb, outtb[:, j * 128 : (j + 1) * 128], ident)
                ob = io_pool.tile([128, 128], F32, tag="ob")
                nc.vector.tensor_copy(ob, ptb)
                nc.sync.dma_start(outB[s, j, :, :], ob)


_CACHE = {}


def _build():
    if "nc" in _CACHE:
        return _CACHE["nc"]
    nc = bacc.Bacc(
        "TRN2",
        target_bir_lowering=False,
        debug=False,
        num_devices=N_CORES,
    )
    ins = {
        "xm": nc.dram_tensor("xm", (PAD, 128), F32, kind="ExternalInput").ap(),
        "xl": nc.dram_tensor("xl", (1, PAD), F32R, kind="ExternalInput").ap(),
    }
    for name, shape in [
        ("w1a", (128, 128)),
        ("wla", (1, 128)),
        ("w1bs", (4, 128, 128)),
        ("wlbs", (4, 128)),
        ("w2a", (128, 128)),
        ("w2b", (128, 128)),
        ("w3a", (128, 128)),
        ("w3b", (128, 128)),
        ("b1a", (128, 1)),
        ("b1b", (128, 1)),
        ("b2a", (128, 1)),
        ("b2b", (128, 1)),
        ("b3a", (128, 1)),
        ("b3b", (128, 1)),
    ]:
        dt = F32R if name.startswith("w") else F32
        ins[name] = nc.dram_tensor(name, shape, dt, kind="ExternalInput").ap()
    outs = {
        "outA": nc.dram_tensor(
            "outA", (N_TILES * 4, 128, 128), F32, kind="ExternalOutput"
        ).ap(),
        "outB": nc.dram_tensor(
            "outB", (N_SUPERS, 4, 128, 128), F32, kind="ExternalOutput"
        ).ap(),
    }
    with tile.TileContext(nc) as tc:
        _kernel_body(tc, outs, ins)
    nc.compile()
    _CACHE["nc"] = nc
    return nc


def kernel(x, W1, b1, W2, b2, W3, b3, _want_trace=False):
    x = np.asarray(x, np.float32)
    wp = _pack_weights(W1, b1, W2, b2, W3, b3)

    x3 = x.reshape(N_CORES, SHARD, IN_DIM)
    in_maps = []
    for c in range(N_CORES):
        xm = np.zeros((PAD, 128), np.float32)
        xm[:SHARD] = x3[c, :, :128]
        xl = np.zeros((1, PAD), np.float32)
        xl[0, :SHARD] = x3[c, :, 128]
        m = {"xm": xm, "xl": xl}
        m.update(wp)
        in_maps.append(m)

    nc = _build()
    res = run_bass_kernel_spmd(
        nc, in_maps, core_ids=list(range(N_CORES)), trace=_want_trace
    )

    out = np.empty((NHEADS, BATCH, SKIP), np.float32)
    for c in range(N_CORES):
        oa = res.results[c]["outA"]  # [496, 128, 128]
        ob = res.results[c]["outB"]  # [31, 4, 128, 128]
        # A: [q, b, (h, o)] -> sample = q*128 + b
        a = oa.reshape(PAD, 8, SKIP).transpose(1, 0, 2)
        out[:8, c * SHARD : (c + 1) * SHARD] = a[:, :SHARD]
        # B: [s, j, b, (cc, g, o)] -> sample = s*2048 + cc*512 + j*128 + b
        b = ob.reshape(N_SUPERS, 4, 128, 4, 2, SKIP)
        b = b.transpose(4, 0, 3, 1, 2, 5).reshape(2, PAD, SKIP)
        out[8:, c * SHARD : (c + 1) * SHARD] = b[:, :SHARD]
    if _want_trace:
        kernel.last_results = res
    return out


# revision 15
# speedup vs baseline: 3.4627x; 3.4627x over previous
"""Trainium2 Bass kernel for nn_EnsembleNet (10-head MLP ensemble).

Math (per head h):
  h1 = relu(x @ W1[h] + b1[h])      x: [B, 129], W1: [129, 16]
  h2 = relu(h1 @ W2[h] + b2[h])     W2: [16, 16]
  out[h] = h2 @ W3[h] + b3[h]       W3: [16, 16] -> [10, B, 16]

Strategy (data parallel over 8 cores, B=500000 -> 62500/core, padded to
63488 = 124 tiles x 512 samples):
  - Host splits x into xm=[B,128] (features 0..127) and xl=[1,B] (feature
    128) so the 129-feature contraction becomes one K=128 matmul plus one
    K=1 accumulate whose moving operand loads directly as a [1, 512] row.
  - Heads 0-7 ("A", 8*16=128 outputs) computed in the transposed domain:
    PE-transpose x tiles to [128 feat, 512 batch], then chained block-
    diagonal matmuls (float32r, 1 cyc/row) with fused bias+relu on ACT,
    PE-transpose the result back to batch-major and DMA out.
  - Heads 8-9 ("B", 32 outputs) are packed 4 sub-tiles deep on the
    partition axis (4 x 32 = 128) per 2048-sample super-tile so vector
    ops and matmuls run at full width; layer 1 uses column-group packing
    (tile_position via psum base partition) to place each sub-tile's
    [K,32] result in its own 32-partition strip.
  - Device writes custom layouts outA [496,128,128] / outB [31,4,128,128];
    host permutes (64B-granular) into [10, B, 16].
"""

import os
from contextlib import ExitStack

import numpy as np

import concourse.bass as bass
import concourse.mybir as mybir
import concourse.tile as tile
from concourse import bacc
from concourse.bass_utils import run_bass_kernel_spmd
from concourse.masks import make_identity

F32 = mybir.dt.float32
F32R = mybir.dt.float32r

N_CORES = 8
BATCH = 500000
SHARD = BATCH // N_CORES  # 62500
TILE = 512  # samples per tile (psum bank = 512 f32)
SUPER = 4 * TILE  # 2048, B-group packing unit
N_TILES = 124  # ceil(62500/512) -> pad to 124
PAD = N_TILES * TILE  # 63488
N_SUPERS = N_TILES // 4  # 31

NHEADS = 10
HID = 16
SKIP = 16
IN_DIM = 129


def _block_diag(mats):
    n = len(mats)
    r, c = mats[0].shape
    out = np.zeros((n * r, n * c), dtype=mats[0].dtype)
    for i, m in enumerate(mats):
        out[i * r : (i + 1) * r, i * c : (i + 1) * c] = m
    return out


def _pack_weights(W1, b1, W2, b2, W3, b3):
    """Host-side packing into the SBUF layouts the kernel expects."""
    W1 = np.asarray(W1, np.float32)
    W2 = np.asarray(W2, np.float32)
    W3 = np.asarray(W3, np.float32)
    b1 = np.asarray(b1, np.float32)
    b2 = np.asarray(b2, np.float32)
    b3 = np.asarray(b3, np.float32)

    d = {}
    # L1 A: lhsT [K=128 feat, M=128 (h,o)]
    d["w1a"] = np.ascontiguousarray(W1[:8, :128, :].transpose(1, 0, 2).reshape(128, 128))
    d["wla"] = np.ascontiguousarray(W1[:8, 128, :].reshape(1, 128))
    # L1 B: zero-padded M=128 variants, chunk c owns columns c*32..(c+1)*32
    # (fp32r matmul requires dst psum base partition 0, so each chunk's
    # [K,32] result is placed via its weight columns instead of col-tiling)
    w1b32 = W1[8:, :128, :].transpose(1, 0, 2).reshape(128, 32)
    wlb32 = W1[8:, 128, :].reshape(32)
    w1bs = np.zeros((4, 128, 128), np.float32)
    wlbs = np.zeros((4, 128), np.float32)
    for c in range(4):
        w1bs[c, :, c * 32 : (c + 1) * 32] = w1b32
        wlbs[c, c * 32 : (c + 1) * 32] = wlb32
    d["w1bs"] = w1bs
    d["wlbs"] = wlbs
    # L2: block diag [in (h,i), out (h,o)]
    d["w2a"] = _block_diag([W2[h] for h in range(8)])
    w2b1 = _block_diag([W2[8], W2[9]])  # [32, 32]
    d["w2b"] = _block_diag([w2b1] * 4)  # [128, 128] over (c, g)
    d["w3a"] = _block_diag([W3[h] for h in range(8)])
    w3b1 = _block_diag([W3[8], W3[9]])
    d["w3b"] = _block_diag([w3b1] * 4)
    # biases, per-partition [128, 1]
    d["b1a"] = b1[:8].reshape(128, 1).copy()
    d["b1b"] = np.tile(b1[8:].reshape(-1), 4).reshape(128, 1)
    d["b2a"] = b2[:8].reshape(128, 1).copy()
    d["b2b"] = np.tile(b2[8:].reshape(-1), 4).reshape(128, 1)
    d["b3a"] = b3[:8].reshape(128, 1).copy()
    d["b3b"] = np.tile(b3[8:].reshape(-1), 4).reshape(128, 1)
    return {k: np.ascontiguousarray(v, dtype=np.float32) for k, v in d.items()}


def _kernel_body(tc, outs, ins, repeat=1):
    nc = tc.nc
    outA, outB = outs["outA"], outs["outB"]  # [496,128,128], [31,4,128,128]
    xm, xl = ins["xm"], ins["xl"]  # [PAD, 128], [1, PAD]
    relu = mybir.ActivationFunctionType.Relu
    copyf = mybir.ActivationFunctionType.Copy

    with ExitStack() as ctx:
        const = ctx.enter_context(tc.tile_pool(name="const", bufs=1))

        ident = const.tile([128, 128], F32)
        make_identity(nc, ident)

        def ld(name, shape, dt=F32):
            t = const.tile(shape, dt, name=name)
            nc.sync.dma_start(t, ins[name])
            return t

        w1a = ld("w1a", [128, 128], F32R)
        wla = ld("wla", [1, 128], F32R)
        w1bs = [None] * 4
        wlbs = [None] * 4
        for c in range(4):
            w1bs[c] = const.tile([128, 128], F32R, name=f"w1bs{c}")
            nc.sync.dma_start(w1bs[c], ins["w1bs"][c])
            wlbs[c] = const.tile([1, 128], F32R, name=f"wlbs{c}")
            nc.sync.dma_start(wlbs[c], ins["wlbs"][c : c + 1, :])
        w2a = ld("w2a", [128, 128], F32R)
        w2b = ld("w2b", [128, 128], F32R)
        w3a = ld("w3a", [128, 128], F32R)
        w3b = ld("w3b", [128, 128], F32R)
        b1a = ld("b1a", [128, 1])
        b1b = ld("b1b", [128, 1])
        b2a = ld("b2a", [128, 1])
        b2b = ld("b2b", [128, 1])
        b3a = ld("b3a", [128, 1])
        b3b = ld("b3b", [128, 1])

        io_pool = ctx.enter_context(tc.tile_pool(name="io", bufs=3))
        xt_pool = ctx.enter_context(tc.tile_pool(name="xt", bufs=6))
        h_pool = ctx.enter_context(tc.tile_pool(name="h", bufs=3))
        bsb_pool = ctx.enter_context(tc.tile_pool(name="bsb", bufs=2))
        xl_pool = ctx.enter_context(tc.tile_pool(name="xlp", bufs=2))
        ppool = ctx.enter_context(tc.tile_pool(name="ppool", space="PSUM", bufs=2))
        papool = ctx.enter_context(tc.tile_pool(name="papool", space="PSUM", bufs=2))
        pbpool = ctx.enter_context(tc.tile_pool(name="pbpool", space="PSUM", bufs=2))
        ptpool = ctx.enter_context(tc.tile_pool(name="ptpool", space="PSUM", bufs=2))

        if repeat > 1:
            # timing-only variant: run the whole body `repeat` times on
            # device so single-dispatch wall time isolates device exec
            ctx.enter_context(tc.For_i(0, repeat, 1))

        for s in range(N_SUPERS):
            # xlast row for this super-tile: [1, 2048] straight from DRAM
            xlt = xl_pool.tile([1, SUPER], F32R, tag="xlt")
            nc.sync.dma_start(xlt, xl[:, s * SUPER : (s + 1) * SUPER])

            pb1 = pbpool.tile([128, TILE], F32, tag="pb")
            xts = []
            for c in range(4):
                t = s * 4 + c
                # natural load: [128 part, (c4, f128)]
                xm_t = io_pool.tile([128, 512], F32, tag="xm")
                src = xm[t * TILE : (t + 1) * TILE, :].rearrange(
                    "(c p) f -> p c f", p=128
                )
                nc.sync.dma_start(xm_t.rearrange("p (c f) -> p c f", c=4), src)

                # transpose 4 chunks of [128,128] -> [feat, batch]
                px = ppool.tile([128, TILE], F32, tag="px")
                for j in range(4):
                    nc.tensor.transpose(
                        px[:, j * 128 : (j + 1) * 128],
                        xm_t[:, j * 128 : (j + 1) * 128],
                        ident,
                    )
                xt = xt_pool.tile([128, TILE], F32R, tag="xt")
                nc.scalar.activation(xt, px, copyf)
                xts.append(xt)

                xls = xlt[:, c * TILE : (c + 1) * TILE]

                # ---- A group ----
                pa1 = papool.tile([128, TILE], F32, tag="pa")
                nc.tensor.matmul(
                    pa1, w1a, xt, start=True, stop=False
                )
                nc.tensor.matmul(
                    pa1, wla, xls, start=False, stop=True
                )
                h1a = h_pool.tile([128, TILE], F32R, tag="h1a")
                nc.scalar.activation(h1a, pa1, relu, bias=b1a)

                pa2 = papool.tile([128, TILE], F32, tag="pa")
                nc.tensor.matmul(
                    pa2, w2a, h1a, start=True, stop=True
                )
                h2a = h_pool.tile([128, TILE], F32R, tag="h2a")
                nc.scalar.activation(h2a, pa2, relu, bias=b2a)

                pa3 = papool.tile([128, TILE], F32, tag="pa")
                nc.tensor.matmul(
                    pa3, w3a, h2a, start=True, stop=True
                )
                outta = h_pool.tile([128, TILE], F32, tag="outta")
                nc.vector.tensor_scalar_add(outta, pa3, b3a)

                # transpose back to batch-major and store
                pt = ptpool.tile([128, TILE], F32, tag="pt")
                for j in range(4):
                    nc.tensor.transpose(
                        pt[:, j * 128 : (j + 1) * 128],
                        outta[:, j * 128 : (j + 1) * 128],
                        ident,
                    )
                oa = io_pool.tile([128, TILE], F32, tag="oa")
                nc.vector.tensor_copy(oa, pt)
                nc.sync.dma_start(
                    outA[t * 4 : (t + 1) * 4, :, :].rearrange("j p f -> p j f"),
                    oa.rearrange("p (j f) -> p j f", j=4),
                )

                # ---- B group layer 1: full-M matmul with zero-padded
                # weights accumulating chunk c's strip into packed psum ----
                nc.tensor.matmul(pb1, w1bs[c], xt, start=(c == 0), stop=False)
                nc.tensor.matmul(pb1, wlbs[c], xls, start=False, stop=(c == 3))

            # ---- B group layers 2..3, packed [128=(c,g,i), 512] ----
            h1b = bsb_pool.tile([128, TILE], F32R, tag="h1b")
            nc.scalar.activation(h1b, pb1, relu, bias=b1b)

            pb2 = pbpool.tile([128, TILE], F32, tag="pb")
            nc.tensor.matmul(
                pb2, w2b, h1b, start=True, stop=True
            )
            h2b = bsb_pool.tile([128, TILE], F32R, tag="h2b")
            nc.scalar.activation(h2b, pb2, relu, bias=b2b)

            pb3 = pbpool.tile([128, TILE], F32, tag="pb")
            nc.tensor.matmul(
                pb3, w3b, h2b, start=True, stop=True
            )
            outtb = bsb_pool.tile([128, TILE], F32, tag="outtb")
            nc.vector.tensor_scalar_add(outtb, pb3, b3b)

            ptb = ptpool.tile([128, TILE], F32, tag="pt")
            for j in range(4):
                nc.tensor.transpose(
                    ptb[:, j * 128 : (j + 1) * 128],
                    outtb[:, j * 128 : (j + 1) * 128],
                    ident,
                )
            ob = io_pool.tile([128, TILE], F32, tag="ob")
            nc.vector.tensor_copy(ob, ptb)
            nc.sync.dma_start(
                outB[s].rearrange("j p f -> p j f"),
                ob.rearrange("p (j f) -> p j f", j=4),
            )


_CACHE = {}


def _build(repeat=1):
    if repeat in _CACHE:
        return _CACHE[repeat]
    nc = bacc.Bacc(
        "TRN2",
        target_bir_lowering=False,
        debug=False,
        num_devices=N_CORES,
    )
    ins = {
        "xm": nc.dram_tensor("xm", (PAD, 128), F32, kind="ExternalInput").ap(),
        "xl": nc.dram_tensor("xl", (1, PAD), F32R, kind="ExternalInput").ap(),
    }
    for name, shape in [
        ("w1a", (128, 128)),
        ("wla", (1, 128)),
        ("w1bs", (4, 128, 128)),
        ("wlbs", (4, 128)),
        ("w2a", (128, 128)),
        ("w2b", (128, 128)),
        ("w3a", (128, 128)),
        ("w3b", (128, 128)),
        ("b1a", (128, 1)),
        ("b1b", (128, 1)),
        ("b2a", (128, 1)),
        ("b2b", (128, 1)),
        ("b3a", (128, 1)),
        ("b3b", (128, 1)),
    ]:
        dt = F32R if name.startswith("w") else F32
        ins[name] = nc.dram_tensor(name, shape, dt, kind="ExternalInput").ap()
    outs = {
        "outA": nc.dram_tensor(
            "outA", (N_TILES * 4, 128, 128), F32, kind="ExternalOutput"
        ).ap(),
        "outB": nc.dram_tensor(
            "outB", (N_SUPERS, 4, 128, 128), F32, kind="ExternalOutput"
        ).ap(),
    }
    with tile.TileContext(nc) as tc:
        _kernel_body(tc, outs, ins, repeat=repeat)
    nc.compile()
    _CACHE[repeat] = nc
    return nc


def kernel(x, W1, b1, W2, b2, W3, b3, _want_trace=False):
    x = np.asarray(x, np.float32)
    wp = _pack_weights(W1, b1, W2, b2, W3, b3)

    x3 = x.reshape(N_CORES, SHARD, IN_DIM)
    in_maps = []
    for c in range(N_CORES):
        xm = np.zeros((PAD, 128), np.float32)
        xm[:SHARD] = x3[c, :, :128]
        xl = np.zeros((1, PAD), np.float32)
        xl[0, :SHARD] = x3[c, :, 128]
        m = {"xm": xm, "xl": xl}
        m.update(wp)
        in_maps.append(m)

    nc = _build()
    res = run_bass_kernel_spmd(
        nc, in_maps, core_ids=list(range(N_CORES)), trace=_want_trace
    )

    out = np.empty((NHEADS, BATCH, SKIP), np.float32)
    for c in range(N_CORES):
        oa = res.results[c]["outA"]  # [496, 128, 128]
        ob = res.results[c]["outB"]  # [31, 4, 128, 128]
        # A: [q, b, (h, o)] -> sample = q*128 + b
        a = oa.reshape(PAD, 8, SKIP).transpose(1, 0, 2)
        out[:8, c * SHARD : (c + 1) * SHARD] = a[:, :SHARD]
        # B: [s, j, b, (cc, g, o)] -> sample = s*2048 + cc*512 + j*128 + b
        b = ob.reshape(N_SUPERS, 4, 128, 4, 2, SKIP)
        b = b.transpose(4, 0, 3, 1, 2, 5).reshape(2, PAD, SKIP)
        out[8:, c * SHARD : (c + 1) * SHARD] = b[:, :SHARD]
    if _want_trace:
        kernel.last_results = res
    return out


# revision 17
# speedup vs baseline: 4.9518x; 1.4300x over previous
"""Trainium2 Bass kernel for nn_EnsembleNet (10-head MLP ensemble).

Math (per head h):
  h1 = relu(x @ W1[h] + b1[h])      x: [B, 129], W1: [129, 16]
  h2 = relu(h1 @ W2[h] + b2[h])     W2: [16, 16]
  out[h] = h2 @ W3[h] + b3[h]       W3: [16, 16] -> [10, B, 16]

Strategy (data parallel over 8 cores, B=500000 -> 62500/core, padded to
63488 = 124 tiles x 512 samples):
  - Host splits x into xm=[B,128] (features 0..127) and xl=[1,B] (feature
    128) so the 129-feature contraction becomes one K=128 matmul plus one
    K=1 accumulate whose moving operand loads directly as a [1, 512] row.
  - Heads 0-7 ("A", 8*16=128 outputs) computed in the transposed domain:
    PE-transpose x tiles to [128 feat, 512 batch], then chained block-
    diagonal matmuls (float32r, 1 cyc/row) with fused bias+relu on ACT,
    PE-transpose the result back to batch-major and DMA out.
  - Heads 8-9 ("B", 32 outputs) are packed 4 sub-tiles deep on the
    partition axis (4 x 32 = 128) per 2048-sample super-tile so vector
    ops and matmuls run at full width; layer 1 uses column-group packing
    (tile_position via psum base partition) to place each sub-tile's
    [K,32] result in its own 32-partition strip.
  - Device writes custom layouts outA [496,128,128] / outB [31,4,128,128];
    host permutes (64B-granular) into [10, B, 16].
"""

import os
from contextlib import ExitStack

import numpy as np

import concourse.bass as bass
import concourse.mybir as mybir
import concourse.tile as tile
from concourse import bacc
from concourse.bass_utils import run_bass_kernel_spmd
from concourse.masks import make_identity

F32 = mybir.dt.float32
F32R = mybir.dt.float32r

N_CORES = 8
BATCH = 500000
SHARD = BATCH // N_CORES  # 62500
TILE = 512  # samples per tile (psum bank = 512 f32)
SUPER = 4 * TILE  # 2048, B-group packing unit
N_TILES = 124  # ceil(62500/512) -> pad to 124
PAD = N_TILES * TILE  # 63488
N_SUPERS = N_TILES // 4  # 31

NHEADS = 10
HID = 16
SKIP = 16
IN_DIM = 129


def _block_diag(mats):
    n = len(mats)
    r, c = mats[0].shape
    out = np.zeros((n * r, n * c), dtype=mats[0].dtype)
    for i, m in enumerate(mats):
        out[i * r : (i + 1) * r, i * c : (i + 1) * c] = m
    return out


def _pack_weights(W1, b1, W2, b2, W3, b3):
    """Host-side packing into the SBUF layouts the kernel expects."""
    W1 = np.asarray(W1, np.float32)
    W2 = np.asarray(W2, np.float32)
    W3 = np.asarray(W3, np.float32)
    b1 = np.asarray(b1, np.float32)
    b2 = np.asarray(b2, np.float32)
    b3 = np.asarray(b3, np.float32)

    d = {}
    # L1 A: lhsT [K=128 feat, M=128 (h,o)]
    d["w1a"] = np.ascontiguousarray(W1[:8, :128, :].transpose(1, 0, 2).reshape(128, 128))
    d["wla"] = np.ascontiguousarray(W1[:8, 128, :].reshape(1, 128))
    # L1 B: zero-padded M=128 variants, chunk c owns columns c*32..(c+1)*32
    # (fp32r matmul requires dst psum base partition 0, so each chunk's
    # [K,32] result is placed via its weight columns instead of col-tiling)
    w1b32 = W1[8:, :128, :].transpose(1, 0, 2).reshape(128, 32)
    wlb32 = W1[8:, 128, :].reshape(32)
    w1bs = np.zeros((4, 128, 128), np.float32)
    wlbs = np.zeros((4, 128), np.float32)
    for c in range(4):
        w1bs[c, :, c * 32 : (c + 1) * 32] = w1b32
        wlbs[c, c * 32 : (c + 1) * 32] = wlb32
    d["w1bs"] = w1bs
    d["wlbs"] = wlbs
    # L2: block diag [in (h,i), out (h,o)]
    d["w2a"] = _block_diag([W2[h] for h in range(8)])
    w2b1 = _block_diag([W2[8], W2[9]])  # [32, 32]
    d["w2b"] = _block_diag([w2b1] * 4)  # [128, 128] over (c, g)
    d["w3a"] = _block_diag([W3[h] for h in range(8)])
    w3b1 = _block_diag([W3[8], W3[9]])
    d["w3b"] = _block_diag([w3b1] * 4)
    # biases, per-partition [128, 1]
    d["b1a"] = b1[:8].reshape(128, 1).copy()
    d["b1b"] = np.tile(b1[8:].reshape(-1), 4).reshape(128, 1)
    d["b2a"] = b2[:8].reshape(128, 1).copy()
    d["b2b"] = np.tile(b2[8:].reshape(-1), 4).reshape(128, 1)
    d["b3a"] = b3[:8].reshape(128, 1).copy()
    d["b3b"] = np.tile(b3[8:].reshape(-1), 4).reshape(128, 1)
    return {k: np.ascontiguousarray(v, dtype=np.float32) for k, v in d.items()}


def _kernel_body(tc, outs, ins, repeat=1):
    nc = tc.nc
    # outputs go on the second HWDGE ring (ACT sequencer) so input and
    # output DMA descriptor streams run in parallel
    odma = nc.scalar if os.environ.get("K_OUT_ENG", "scalar") == "scalar" else nc.sync
    strip = os.environ.get("K_STRIP", "")
    outA, outB = outs["outA"], outs["outB"]  # [496,128,128], [31,4,128,128]
    xm, xl = ins["xm"], ins["xl"]  # [PAD, 128], [1, PAD]
    relu = mybir.ActivationFunctionType.Relu
    copyf = mybir.ActivationFunctionType.Copy

    with ExitStack() as ctx:
        const = ctx.enter_context(tc.tile_pool(name="const", bufs=1))

        ident = const.tile([128, 128], F32)
        make_identity(nc, ident)

        def ld(name, shape, dt=F32):
            t = const.tile(shape, dt, name=name)
            nc.sync.dma_start(t, ins[name])
            return t

        w1a = ld("w1a", [128, 128], F32R)
        wla = ld("wla", [1, 128], F32R)
        w1bs = [None] * 4
        wlbs = [None] * 4
        for c in range(4):
            w1bs[c] = const.tile([128, 128], F32R, name=f"w1bs{c}")
            nc.sync.dma_start(w1bs[c], ins["w1bs"][c])
            wlbs[c] = const.tile([1, 128], F32R, name=f"wlbs{c}")
            nc.sync.dma_start(wlbs[c], ins["wlbs"][c : c + 1, :])
        w2a = ld("w2a", [128, 128], F32R)
        w2b = ld("w2b", [128, 128], F32R)
        w3a = ld("w3a", [128, 128], F32R)
        w3b = ld("w3b", [128, 128], F32R)
        b1a = ld("b1a", [128, 1])
        b1b = ld("b1b", [128, 1])
        b2a = ld("b2a", [128, 1])
        b2b = ld("b2b", [128, 1])
        b3a = ld("b3a", [128, 1])
        b3b = ld("b3b", [128, 1])

        io_pool = ctx.enter_context(tc.tile_pool(name="io", bufs=3))
        xt_pool = ctx.enter_context(tc.tile_pool(name="xt", bufs=6))
        h_pool = ctx.enter_context(tc.tile_pool(name="h", bufs=3))
        bsb_pool = ctx.enter_context(tc.tile_pool(name="bsb", bufs=2))
        xl_pool = ctx.enter_context(tc.tile_pool(name="xlp", bufs=2))
        ppool = ctx.enter_context(tc.tile_pool(name="ppool", space="PSUM", bufs=2))
        papool = ctx.enter_context(tc.tile_pool(name="papool", space="PSUM", bufs=2))
        pbpool = ctx.enter_context(tc.tile_pool(name="pbpool", space="PSUM", bufs=2))
        ptpool = ctx.enter_context(tc.tile_pool(name="ptpool", space="PSUM", bufs=2))

        if repeat > 1:
            # timing-only variant: run the whole body `repeat` times on
            # device so single-dispatch wall time isolates device exec
            ctx.enter_context(tc.For_i(0, repeat, 1))

        if strip == "dmaonly":
            for s in range(N_SUPERS):
                for c in range(4):
                    t = s * 4 + c
                    xm_t = io_pool.tile([128, 512], F32, tag="xm")
                    src_ap = xm[t * TILE : (t + 1) * TILE, :].rearrange(
                        "(c p) f -> p c f", p=128
                    )
                    nc.sync.dma_start(xm_t.rearrange("p (c f) -> p c f", c=4), src_ap)
                    odma.dma_start(
                        outA[t * 4 : (t + 1) * 4, :, :].rearrange("j p f -> p j f"),
                        xm_t.rearrange("p (j f) -> p j f", j=4),
                    )
                ob = io_pool.tile([128, TILE], F32, tag="ob")
                nc.vector.tensor_copy(ob, xm_t)
                odma.dma_start(
                    outB[s].rearrange("j p f -> p j f"),
                    ob.rearrange("p (j f) -> p j f", j=4),
                )
            return

        for s in range(N_SUPERS):
            # xlast row for this super-tile: [1, 2048] straight from DRAM
            xlt = xl_pool.tile([1, SUPER], F32R, tag="xlt")
            nc.sync.dma_start(xlt, xl[:, s * SUPER : (s + 1) * SUPER])

            pb1 = pbpool.tile([128, TILE], F32, tag="pb")
            xts = []
            for c in range(4):
                t = s * 4 + c
                # natural load: [128 part, (c4, f128)]
                xm_t = io_pool.tile([128, 512], F32, tag="xm")
                src = xm[t * TILE : (t + 1) * TILE, :].rearrange(
                    "(c p) f -> p c f", p=128
                )
                nc.sync.dma_start(xm_t.rearrange("p (c f) -> p c f", c=4), src)

                # transpose 4 chunks of [128,128] -> [feat, batch]
                if strip == "notrans":
                    xt = xt_pool.tile([128, TILE], F32R, tag="xt")
                    nc.scalar.activation(xt, xm_t, copyf)
                else:
                    px = ppool.tile([128, TILE], F32, tag="px")
                    for j in range(4):
                        nc.tensor.transpose(
                            px[:, j * 128 : (j + 1) * 128],
                            xm_t[:, j * 128 : (j + 1) * 128],
                            ident,
                        )
                    xt = xt_pool.tile([128, TILE], F32R, tag="xt")
                    nc.scalar.activation(xt, px, copyf)
                xts.append(xt)

                xls = xlt[:, c * TILE : (c + 1) * TILE]

                # ---- A group ----
                pa1 = papool.tile([128, TILE], F32, tag="pa")
                nc.tensor.matmul(
                    pa1, w1a, xt, start=True, stop=False
                )
                nc.tensor.matmul(
                    pa1, wla, xls, start=False, stop=True
                )
                h1a = h_pool.tile([128, TILE], F32R, tag="h1a")
                nc.scalar.activation(h1a, pa1, relu, bias=b1a)

                pa2 = papool.tile([128, TILE], F32, tag="pa")
                nc.tensor.matmul(
                    pa2, w2a, h1a, start=True, stop=True
                )
                h2a = h_pool.tile([128, TILE], F32R, tag="h2a")
                nc.scalar.activation(h2a, pa2, relu, bias=b2a)

                pa3 = papool.tile([128, TILE], F32, tag="pa")
                nc.tensor.matmul(
                    pa3, w3a, h2a, start=True, stop=True
                )
                outta = h_pool.tile([128, TILE], F32, tag="outta")
                nc.vector.tensor_scalar_add(outta, pa3, b3a)

                # transpose back to batch-major and store
                if strip == "notrans":
                    oa = outta
                else:
                    pt = ptpool.tile([128, TILE], F32, tag="pt")
                    for j in range(4):
                        nc.tensor.transpose(
                            pt[:, j * 128 : (j + 1) * 128],
                            outta[:, j * 128 : (j + 1) * 128],
                            ident,
                        )
                    oa = io_pool.tile([128, TILE], F32, tag="oa")
                    nc.vector.tensor_copy(oa, pt)
                odma.dma_start(
                    outA[t * 4 : (t + 1) * 4, :, :].rearrange("j p f -> p j f"),
                    oa.rearrange("p (j f) -> p j f", j=4),
                )

                # ---- B group layer 1: full-M matmul with zero-padded
                # weights accumulating chunk c's strip into packed psum ----
                nc.tensor.matmul(pb1, w1bs[c], xt, start=(c == 0), stop=False)
                nc.tensor.matmul(pb1, wlbs[c], xls, start=False, stop=(c == 3))

            # ---- B group layers 2..3, packed [128=(c,g,i), 512] ----
            h1b = bsb_pool.tile([128, TILE], F32R, tag="h1b")
            nc.scalar.activation(h1b, pb1, relu, bias=b1b)

            pb2 = pbpool.tile([128, TILE], F32, tag="pb")
            nc.tensor.matmul(
                pb2, w2b, h1b, start=True, stop=True
            )
            h2b = bsb_pool.tile([128, TILE], F32R, tag="h2b")
            nc.scalar.activation(h2b, pb2, relu, bias=b2b)

            pb3 = pbpool.tile([128, TILE], F32, tag="pb")
            nc.tensor.matmul(
                pb3, w3b, h2b, start=True, stop=True
            )
            outtb = bsb_pool.tile([128, TILE], F32, tag="outtb")
            nc.vector.tensor_scalar_add(outtb, pb3, b3b)

            if strip == "notrans":
                ob = outtb
            else:
                ptb = ptpool.tile([128, TILE], F32, tag="pt")
                for j in range(4):
                    nc.tensor.transpose(
                        ptb[:, j * 128 : (j + 1) * 128],
                        outtb[:, j * 128 : (j + 1) * 128],
                        ident,
                    )
                ob = io_pool.tile([128, TILE], F32, tag="ob")
                nc.vector.tensor_copy(ob, ptb)
            odma.dma_start(
                outB[s].rearrange("j p f -> p j f"),
                ob.rearrange("p (j f) -> p j f", j=4),
            )


_CACHE = {}


def _build(repeat=1):
    key = (repeat, os.environ.get("K_OUT_ENG", "scalar"), os.environ.get("K_STRIP", ""))
    if key in _CACHE:
        return _CACHE[key]
    nc = bacc.Bacc(
        "TRN2",
        target_bir_lowering=False,
        debug=False,
        num_devices=N_CORES,
    )
    ins = {
        "xm": nc.dram_tensor("xm", (PAD, 128), F32, kind="ExternalInput").ap(),
        "xl": nc.dram_tensor("xl", (1, PAD), F32R, kind="ExternalInput").ap(),
    }
    for name, shape in [
        ("w1a", (128, 128)),
        ("wla", (1, 128)),
        ("w1bs", (4, 128, 128)),
        ("wlbs", (4, 128)),
        ("w2a", (128, 128)),
        ("w2b", (128, 128)),
        ("w3a", (128, 128)),
        ("w3b", (128, 128)),
        ("b1a", (128, 1)),
        ("b1b", (128, 1)),
        ("b2a", (128, 1)),
        ("b2b", (128, 1)),
        ("b3a", (128, 1)),
        ("b3b", (128, 1)),
    ]:
        dt = F32R if name.startswith("w") else F32
        ins[name] = nc.dram_tensor(name, shape, dt, kind="ExternalInput").ap()
    outs = {
        "outA": nc.dram_tensor(
            "outA", (N_TILES * 4, 128, 128), F32, kind="ExternalOutput"
        ).ap(),
        "outB": nc.dram_tensor(
            "outB", (N_SUPERS, 4, 128, 128), F32, kind="ExternalOutput"
        ).ap(),
    }
    with tile.TileContext(nc) as tc:
        _kernel_body(tc, outs, ins, repeat=repeat)
    nc.compile()
    _CACHE[key] = nc
    return nc


def kernel(x, W1, b1, W2, b2, W3, b3, _want_trace=False):
    x = np.asarray(x, np.float32)
    wp = _pack_weights(W1, b1, W2, b2, W3, b3)

    x3 = x.reshape(N_CORES, SHARD, IN_DIM)
    in_maps = []
    for c in range(N_CORES):
        xm = np.zeros((PAD, 128), np.float32)
        xm[:SHARD] = x3[c, :, :128]
        xl = np.zeros((1, PAD), np.float32)
        xl[0, :SHARD] = x3[c, :, 128]
        m = {"xm": xm, "xl": xl}
        m.update(wp)
        in_maps.append(m)

    nc = _build()
    res = run_bass_kernel_spmd(
        nc, in_maps, core_ids=list(range(N_CORES)), trace=_want_trace
    )

    out = np.empty((NHEADS, BATCH, SKIP), np.float32)
    for c in range(N_CORES):
        oa = res.results[c]["outA"]  # [496, 128, 128]
        ob = res.results[c]["outB"]  # [31, 4, 128, 128]
        # A: [q, b, (h, o)] -> sample = q*128 + b
        a = oa.reshape(PAD, 8, SKIP).transpose(1, 0, 2)
        out[:8, c * SHARD : (c + 1) * SHARD] = a[:, :SHARD]
        # B: [s, j, b, (cc, g, o)] -> sample = s*2048 + cc*512 + j*128 + b
        b = ob.reshape(N_SUPERS, 4, 128, 4, 2, SKIP)
        b = b.transpose(4, 0, 3, 1, 2, 5).reshape(2, PAD, SKIP)
        out[8:, c * SHARD : (c + 1) * SHARD] = b[:, :SHARD]
    if _want_trace:
        kernel.last_results = res
    return out
